# revision 9
# baseline (speedup 1.0000x reference)
"""Trainium2 Bass kernel for nn_DVGMatchModule (dense transformer, 8 cores).

Contract: kernel(**inputs) takes the FULL unsharded numpy inputs (keys as in
setup_inputs()) and returns the FULL (B*M, K) float32 output.

Sharding: data-parallel over B — core c handles batches {2c, 2c+1}, i.e. 32 of
the 256 effective sequences. All weights replicated. No collectives.

Device layout strategy:
  - Activations feature-major (HID=128 on partitions, tokens on free dim).
  - Attention in transposed orientation: S^T = K_h^T Q_h with keys on
    partitions, so the softmax denominator is a ones-vector matmul, the
    distance-bias normalization is a free-dim reduce, and the CA key mask is a
    per-partition ACT bias.
  - Algebraic folds (host-side, exact): SCALE into Wq/bq; bk dropped (softmax
    shift invariance); bv folded into bo_eff = bo + Wo@bv (softmax rows sum to
    1); BN+PReLU fused into one ACT Prelu (scale/bias/alpha APs); conv biases
    folded into BN shift.
"""
import os
from contextlib import ExitStack

import numpy as np

B, K, M, L = 16, 256, 16, 64
DET_C, HID, H, DEPTH = 1152, 128, 4, 2
DK = HID // H
SCALE = np.float32(1.0 / np.sqrt(DK))
EPS = 1e-5
NCORES = 8
NB = B // NCORES          # batches per core
NSEQ = NB * M             # sequences per core
NG = NSEQ // 4            # 4-seq lang groups per core
KC = K // 128             # q/k chunks of 128 tokens

F32 = None  # set lazily (mybir.dt.float32)

_COMPILED = {}


def _f32(x):
    return np.ascontiguousarray(np.asarray(x, np.float32))


def _prep_host(inp):
    """Host-side weight prep + per-core input shards. Pure layout/fold work."""
    g = {}
    s1 = _f32(inp['bn1_g']) / np.sqrt(_f32(inp['bn1_v']) + EPS)
    t1 = _f32(inp['bn1_b']) - _f32(inp['bn1_m']) * s1 + _f32(inp['fc_b1']) * s1
    g['fc_W1T'] = _f32(inp['fc_W1']).T.reshape(9, 128, 128)
    g['fc_s1'] = s1.reshape(128, 1)
    g['fc_t1'] = t1.reshape(128, 1)
    g['pr1_a'] = _f32(inp['pr1_a']).reshape(128, 1)
    g['fc_W2T'] = _f32(inp['fc_W2']).T
    g['fc_b2'] = _f32(inp['fc_b2']).reshape(128, 1)
    for pre in ['sa', 'ca']:
        for i in range(DEPTH):
            Wq = _f32(inp[f'{pre}_Wq'][i]); Wo = _f32(inp[f'{pre}_Wo'][i])
            g[f'{pre}{i}_WqT'] = (Wq * SCALE).T.copy()
            g[f'{pre}{i}_bq'] = (_f32(inp[f'{pre}_bq'][i]) * SCALE).reshape(128, 1)
            g[f'{pre}{i}_WkT'] = _f32(inp[f'{pre}_Wk'][i]).T.copy()
            g[f'{pre}{i}_WvT'] = _f32(inp[f'{pre}_Wv'][i]).T.copy()
            g[f'{pre}{i}_WoT'] = Wo.T.copy()
            g[f'{pre}{i}_bo'] = (_f32(inp[f'{pre}_bo'][i]) + Wo @ _f32(inp[f'{pre}_bv'][i])).reshape(128, 1)
            g[f'{pre}{i}_lg'] = _f32(inp[f'{pre}_lg'][i]).reshape(1, 128)
            g[f'{pre}{i}_lb'] = _f32(inp[f'{pre}_lb'][i]).reshape(1, 128)
    ms1 = _f32(inp['mbn1_g']) / np.sqrt(_f32(inp['mbn1_v']) + EPS)
    mt1 = _f32(inp['mbn1_b']) - _f32(inp['mbn1_m']) * ms1 + _f32(inp['m_b1']) * ms1
    ms2 = _f32(inp['mbn2_g']) / np.sqrt(_f32(inp['mbn2_v']) + EPS)
    mt2 = _f32(inp['mbn2_b']) - _f32(inp['mbn2_m']) * ms2 + _f32(inp['m_b2']) * ms2
    g['m_W1T'] = _f32(inp['m_W1']).T.copy()
    g['m_s1'] = ms1.reshape(128, 1); g['m_t1'] = mt1.reshape(128, 1)
    g['m_W2T'] = _f32(inp['m_W2']).T.copy()
    g['m_s2'] = ms2.reshape(128, 1); g['m_t2'] = mt2.reshape(128, 1)
    g['m_W3T'] = _f32(inp['m_W3']).T.copy()            # (128,1)
    g['m_b3'] = _f32(inp['m_b3']).reshape(1, 1)
    g['mp1_a'] = np.full((128, 1), float(np.asarray(inp['mp1_a']).reshape(-1)[0]), np.float32)
    g['mp2_a'] = np.full((128, 1), float(np.asarray(inp['mp2_a']).reshape(-1)[0]), np.float32)

    center = _f32(inp['center'])                        # (B,256,3)
    detr = _f32(inp['detr_features'])                   # (B,256,1152)
    lang = _f32(inp['lang_fea'])                        # (B*M,64,128)
    mask_neg = np.where(np.asarray(inp['attention_mask'])[:, 0, 0, :],
                        np.float32(-100.0), np.float32(0.0))  # (B*M,64)

    shards = []
    for c in range(NCORES):
        bs = [NB * c + j for j in range(NB)]
        s0 = bs[0] * M
        sh = {}
        sh['centerT'] = np.ascontiguousarray(center[bs].transpose(0, 2, 1))        # (NB,3,256)
        sh['detrT'] = np.ascontiguousarray(
            detr[bs].transpose(0, 2, 1).reshape(NB, 9, 128, 256))                  # (NB,9,128,256)
        lg = lang[s0:s0 + NSEQ].transpose(0, 2, 1).reshape(NG, 4, 128, 64)
        sh['langT'] = np.ascontiguousarray(lg.transpose(0, 2, 1, 3).reshape(NG, 128, 256))
        sh['maskT'] = np.ascontiguousarray(mask_neg[s0:s0 + NSEQ].T)               # (64,NSEQ)
        shards.append(sh)
    return g, shards


# ---------------------------------------------------------------------------
# Device kernel builder
# ---------------------------------------------------------------------------

def _build(nc, tc, ctx, io, mm_np=False):
    """Emit the whole per-core program. io: dict name->AP of dram tensors.

    mm_np: if True keep matmuls in plain float32 (4 cyc/row); else bitcast
    matmul operands to float32r (1 cyc/row at N>=256).
    """
    import concourse.bass as bass
    import concourse.mybir as mybir
    from concourse.masks import make_identity

    dt = mybir.dt
    AF = mybir.ActivationFunctionType
    ALU = mybir.AluOpType

    def r(ap):  # matmul-operand dtype view
        return ap if mm_np else ap.bitcast(dt.float32r)

    wpool = ctx.enter_context(tc.tile_pool(name="weights", bufs=1))
    persist = ctx.enter_context(tc.tile_pool(name="persist", bufs=1))
    act = ctx.enter_context(tc.tile_pool(name="act", bufs=3))
    small = ctx.enter_context(tc.tile_pool(name="small", bufs=4))
    lpool = ctx.enter_context(tc.tile_pool(name="lang", bufs=2))
    # PSUM: 8 banks total; every tile slot occupies a full bank. 3 tags only.
    psA = ctx.enter_context(tc.tile_pool(name="psA", bufs=3, space="PSUM"))
    psB = ctx.enter_context(tc.tile_pool(name="psB", bufs=3, space="PSUM"))
    psC = ctx.enter_context(tc.tile_pool(name="psC", bufs=2, space="PSUM"))

    def psum(shape, tag=None):
        return psA.tile(list(shape), mybir.dt.float32, tag="mm", name="ps_mm")

    def psum_t(shape):
        return psB.tile(list(shape), mybir.dt.float32, tag="tp", name="ps_tp")

    def psum_c(shape):
        return psC.tile(list(shape), mybir.dt.float32, tag="se", name="ps_se")

    # ---- constants / weights into SBUF ----
    ident = wpool.tile([128, 128], dt.float32)
    make_identity(nc, ident)
    ones128 = wpool.tile([128, 1], dt.float32)
    nc.vector.memset(ones128, 1.0)
    ones_blk = wpool.tile([128, 32], dt.float32)
    nc.vector.memset(ones_blk, 1.0)
    ones_row = wpool.tile([1, 128], dt.float32)
    nc.vector.memset(ones_row, 1.0)
    ones3 = wpool.tile([3, 1], dt.float32)
    nc.vector.memset(ones3, 1.0)
    eps_t = wpool.tile([128, 1], dt.float32)
    nc.vector.memset(eps_t, EPS)

    def w_load(name, shape):
        t = wpool.tile(list(shape), dt.float32, tag=f"w_{name}", name=f"w_{name}")
        nc.sync.dma_start(out=t, in_=io[name])
        return t

    def w_bcast(name):  # (1,128) dram -> (128,128) sbuf broadcast over partitions
        t = wpool.tile([128, 128], dt.float32, tag=f"w_{name}", name=f"w_{name}")
        src = io[name]
        bap = bass.AP(tensor=src.tensor, offset=src.offset,
                      ap=[[0, 128]] + list(src.ap[1:]))
        nc.gpsimd.dma_start(out=t, in_=bap)
        return t

    W = {}
    W['fc_W1T'] = wpool.tile([128, 9, 128], dt.float32, tag='w_fc_W1T', name='w_fc_W1T')
    nc.sync.dma_start(out=W['fc_W1T'],
                      in_=io['fc_W1T'].rearrange("a b c -> b a c"))
    for nm in ['fc_s1', 'fc_t1', 'pr1_a', 'fc_b2', 'm_s1', 'm_t1', 'm_s2',
               'm_t2', 'm_W3T', 'mp1_a', 'mp2_a']:
        W[nm] = w_load(nm, (128, 1))
    for nm in ['fc_W2T', 'm_W1T', 'm_W2T']:
        W[nm] = w_load(nm, (128, 128))
    W['m_b3'] = w_load('m_b3', (1, 1))
    for pre in ['sa', 'ca']:
        for i in range(DEPTH):
            p = f'{pre}{i}'
            for nm in ['WqT', 'WkT', 'WvT', 'WoT']:
                W[f'{p}_{nm}'] = w_load(f'{p}_{nm}', (128, 128))
            for nm in ['bq', 'bo']:
                W[f'{p}_{nm}'] = w_load(f'{p}_{nm}', (128, 1))
            W[f'{p}_lg'] = w_bcast(f'{p}_lg')
            W[f'{p}_lb'] = w_bcast(f'{p}_lb')

    maskT = wpool.tile([64, NSEQ], dt.float32)
    nc.sync.dma_start(out=maskT, in_=io['maskT'])

    # ---- per-b persistent tiles ----
    bias_dw = persist.tile([128, NB, KC, 256], dt.float32)   # T0 (dw-normalized)
    bias_nd = persist.tile([128, NB, KC, 256], dt.float32)   # dist (subtract it)
    feats_t = persist.tile([128, NB, 256], dt.float32)
    q0_t = persist.tile([128, NB, 256], dt.float32)

    out_dram = io['out']

    # ------------------------------------------------------------------
    def ln_apply(r_fm_sb, lg_bc, lb_bc, out_fm):
        """r (128,256) fm -> LN over features -> out_fm (128,256) sbuf."""
        for qc in range(KC):
            rtm = psum_t((128, 128))
            nc.tensor.transpose(rtm, r_fm_sb[:, qc * 128:(qc + 1) * 128], ident)
            stats = small.tile([128, 6], dt.float32, tag="stats")
            nc.vector.bn_stats(out=stats, in_=rtm)
            mv = small.tile([128, 2], dt.float32, tag="mv")
            nc.vector.bn_aggr(out=mv, in_=stats)
            rstd = small.tile([128, 1], dt.float32, tag="rstd")
            nc.scalar.activation(out=rstd, in_=mv[:, 1:2], func=AF.Sqrt, bias=eps_t)
            nc.vector.reciprocal(out=rstd, in_=rstd)
            nmr = small.tile([128, 1], dt.float32, tag="nmr")
            nc.vector.tensor_scalar(out=nmr, in0=mv[:, 0:1], scalar1=rstd,
                                    scalar2=-1.0, op0=ALU.mult, op1=ALU.mult)
            xn = act.tile([128, 128], dt.float32, tag="xn")
            nc.scalar.activation(out=xn, in_=rtm, func=AF.Identity,
                                 bias=nmr, scale=rstd)
            xg = act.tile([128, 128], dt.float32, tag="xg")
            nc.vector.tensor_mul(xg, xn, lg_bc)
            nc.vector.tensor_add(xg, xg, lb_bc)
            xf = psum_t((128, 128))
            nc.tensor.transpose(xf, xg, ident)
            nc.vector.tensor_copy(out_fm[:, qc * 128:(qc + 1) * 128], xf)

    def attention(X_fm, Ksrc_fm, pre, i, Tk, bi=None, mask_sc=None, Q_pre=None,
                  out_fm=None):
        """MHA + residual + LN. X_fm (128,256) sbuf; Ksrc_fm (128,Tk) sbuf.

        bi: batch index -> use SA bias tiles. mask_sc: seq index -> CA mask col.
        Q_pre: precomputed Q (128,256) sbuf. Returns out_fm (128,256) sbuf.
        """
        p = f'{pre}{i}'
        kc = Tk // 128 if Tk >= 128 else 1
        kpart = min(Tk, 128)

        if Q_pre is None:
            qp = psum((128, 256))
            nc.tensor.matmul(qp, r(W[f'{p}_WqT']), r(X_fm), start=True, stop=True)
            Q = act.tile([128, 256], dt.float32, tag="Q")
            nc.scalar.activation(out=Q, in_=qp, func=AF.Identity, bias=W[f'{p}_bq'])
        else:
            Q = Q_pre
        kp = psum((128, 256))
        nc.tensor.matmul(kp[:, :Tk], r(W[f'{p}_WkT']), r(Ksrc_fm), start=True, stop=True)
        Kf = act.tile([128, 256], dt.float32, tag="Kf")
        nc.vector.tensor_copy(Kf[:, :Tk], kp[:, :Tk])
        vp = psum((128, 256))
        nc.tensor.matmul(vp[:, :Tk], r(W[f'{p}_WvT']), r(Ksrc_fm), start=True, stop=True)
        Vf = act.tile([128, 256], dt.float32, tag="Vf")
        nc.vector.tensor_copy(Vf[:, :Tk], vp[:, :Tk])
        # V token-major via PE transpose
        Vtm = act.tile([128, 2, 128], dt.float32, tag="Vtm")  # (kpart, kc, 128)
        for c in range(kc):
            vt = psum_t((128, 128))
            nc.tensor.transpose(vt[:kpart, :], Vf[:, c * 128:c * 128 + kpart], ident)
            nc.vector.tensor_copy(Vtm[:kpart, c, :], vt[:kpart, :])

        E = act.tile([128, H, 2, 256], dt.float32, tag="E")   # (kpart, h, kc, q)
        SE = psum((128, 256))
        for h in range(H):
            hs = slice(h * DK, (h + 1) * DK)
            for c in range(kc):
                st = psum((128, 256))
                nc.tensor.matmul(st[:kpart, :],
                                 r(Kf[hs, c * 128:c * 128 + kpart]),
                                 r(Q[hs, :]), start=True, stop=True,
                                 tile_position=(h * DK, 0))
                if bi is not None and h == 0:
                    lg_t = act.tile([128, 256], dt.float32, tag="lg_t")
                    nc.vector.tensor_add(lg_t, st, bias_dw[:, bi, c, :])
                    nc.scalar.activation(out=E[:, h, c, :], in_=lg_t, func=AF.Exp)
                elif bi is not None and h == 1:
                    lg_t = act.tile([128, 256], dt.float32, tag="lg_t")
                    nc.vector.tensor_sub(lg_t, st, bias_nd[:, bi, c, :])
                    nc.scalar.activation(out=E[:, h, c, :], in_=lg_t, func=AF.Exp)
                elif mask_sc is not None:
                    nc.scalar.activation(out=E[:kpart, h, c, :], in_=st[:kpart, :],
                                         func=AF.Exp, bias=maskT[:, mask_sc:mask_sc + 1])
                else:
                    nc.scalar.activation(out=E[:, h, c, :], in_=st, func=AF.Exp)
                nc.tensor.matmul(SE[h * DK:(h + 1) * DK, :], r(ones_blk[:kpart, :]),
                                 r(E[:kpart, h, c, :]),
                                 start=(c == 0), stop=(c == kc - 1),
                                 tile_position=(0, h * DK))
        SEs = act.tile([128, 256], dt.float32, tag="SEs")
        nc.scalar.copy(SEs, SE)
        R = small.tile([128, 2, 4], dt.float32, tag="R")
        for qc in range(KC):
            rt = psum_t((128, 128))
            nc.tensor.transpose(rt, SEs[:, qc * 128:(qc + 1) * 128], ident)
            nc.vector.reciprocal(R[:, qc, :], rt.rearrange("p (h d) -> p h d", d=DK)[:, :, 0])
        # O token-major, scaled
        Os = act.tile([128, 2, 128], dt.float32, tag="Os")    # (q, qc, feat)
        for qc in range(KC):
            ot = psum_t((128, 128))
            for h in range(H):
                for c in range(kc):
                    nc.tensor.matmul(ot[:, h * DK:(h + 1) * DK],
                                     r(E[:kpart, h, c, qc * 128:(qc + 1) * 128]),
                                     r(Vtm[:kpart, c, h * DK:(h + 1) * DK]),
                                     start=(c == 0), stop=(c == kc - 1))
            for h in range(H):
                nc.vector.tensor_scalar_mul(Os[:, qc, h * DK:(h + 1) * DK],
                                            ot[:, h * DK:(h + 1) * DK],
                                            R[:, qc, h:h + 1])
        Of = act.tile([128, 256], dt.float32, tag="Of")
        for qc in range(KC):
            op = psum_t((128, 128))
            nc.tensor.transpose(op, Os[:, qc, :], ident)
            nc.vector.tensor_copy(Of[:, qc * 128:(qc + 1) * 128], op)
        ap = psum((128, 256))
        nc.tensor.matmul(ap, r(W[f'{p}_WoT']), r(Of), start=True, stop=True)
        rr = act.tile([128, 256], dt.float32, tag="rr")
        nc.vector.scalar_tensor_tensor(out=rr, in0=ap, scalar=W[f'{p}_bo'],
                                       in1=X_fm, op0=ALU.add, op1=ALU.add)
        if out_fm is None:
            out_fm = act.tile([128, 256], dt.float32, tag="attn_out")
        ln_apply(rr, W[f'{p}_lg'], W[f'{p}_lb'], out_fm)
        return out_fm

    # ------------------------------------------------------------------
    # Per-batch phase: distance bias, fc stage, SA0, CA0-Q precompute
    # ------------------------------------------------------------------
    for bi in range(NB):
        Ct = small.tile([3, 256], dt.float32, tag="Ct")
        nc.sync.dma_start(out=Ct, in_=io['centerT'][bi])
        Csq = small.tile([3, 256], dt.float32, tag="Csq")
        nc.scalar.activation(out=Csq, in_=Ct, func=AF.Square)
        Cm2 = small.tile([3, 256], dt.float32, tag="Cm2")
        nc.scalar.mul(Cm2, Ct, -2.0)
        nrow_p = psum_c((1, 256))
        nc.tensor.matmul(nrow_p, r(ones3), r(Csq), start=True, stop=True)
        nrow = small.tile([1, 256], dt.float32, tag="nrow")
        nc.vector.tensor_copy(nrow, nrow_p)
        for qc in range(KC):
            ncol_p = psum_c((128, 1))
            nc.tensor.matmul(ncol_p, r(Csq[:, qc * 128:(qc + 1) * 128]), r(ones3),
                             start=True, stop=True)
            dp = psum((128, 256))
            nc.tensor.matmul(dp, r(Ct[:, qc * 128:(qc + 1) * 128]), r(Cm2),
                             start=True, stop=False)
            nc.tensor.matmul(dp, r(ones_row), r(nrow), start=False, stop=True)
            d2 = act.tile([128, 256], dt.float32, tag="d2")
            nc.vector.tensor_scalar(out=d2, in0=dp, scalar1=ncol_p, scalar2=0.0,
                                    op0=ALU.add, op1=ALU.max)
            dist = bias_nd[:, bi, qc, :]
            nc.scalar.activation(out=dist, in_=d2, func=AF.Sqrt)
            dwt = act.tile([128, 256], dt.float32, tag="dwt")
            nc.vector.tensor_scalar_add(dwt, dist, 0.01)
            nc.vector.reciprocal(dwt, dwt)
            rs = small.tile([128, 1], dt.float32, tag="rs")
            nc.vector.reduce_sum(out=rs, in_=dwt, axis=mybir.AxisListType.X)
            nc.vector.reciprocal(rs, rs)
            nc.vector.tensor_scalar_mul(bias_dw[:, bi, qc, :], dwt, rs)

        # fc stage
        yp = psum((128, 256))
        for j in range(9):
            dch = act.tile([128, 256], dt.float32, tag="dch")
            nc.sync.dma_start(out=dch, in_=io['detrT'][bi, j])
            nc.tensor.matmul(yp, r(W['fc_W1T'][:, j, :]), r(dch),
                             start=(j == 0), stop=(j == 8))
        h1 = act.tile([128, 256], dt.float32, tag="h1")
        nc.scalar.activation(out=h1, in_=yp, func=AF.Prelu,
                             bias=W['fc_t1'], scale=W['fc_s1'], alpha=W['pr1_a'])
        xp = psum((128, 256))
        nc.tensor.matmul(xp, r(W['fc_W2T']), r(h1), start=True, stop=True)
        X0 = act.tile([128, 256], dt.float32, tag="X0")
        nc.scalar.activation(out=X0, in_=xp, func=AF.Identity, bias=W['fc_b2'])

        attention(X0, X0, 'sa', 0, Tk=256, bi=bi, out_fm=feats_t[:, bi, :])
        q0p = psum((128, 256))
        nc.tensor.matmul(q0p, r(W['ca0_WqT']), r(feats_t[:, bi, :]),
                         start=True, stop=True)
        nc.scalar.activation(out=q0_t[:, bi, :], in_=q0p, func=AF.Identity,
                             bias=W['ca0_bq'])

    # ------------------------------------------------------------------
    # Per-sequence main loop
    # ------------------------------------------------------------------
    for sc in range(NSEQ):
        bi, m = divmod(sc, M)
        if sc % 4 == 0:
            lang4 = lpool.tile([128, 256], dt.float32, tag="lang4")
            nc.sync.dma_start(out=lang4, in_=io['langT'][sc // 4])
        langT = lang4[:, (sc % 4) * 64:(sc % 4) * 64 + 64]

        f1 = attention(feats_t[:, bi, :], langT, 'ca', 0, Tk=64,
                       mask_sc=sc, Q_pre=q0_t[:, bi, :])
        f1 = attention(f1, f1, 'sa', 1, Tk=256, bi=bi)
        f1 = attention(f1, langT, 'ca', 1, Tk=64, mask_sc=sc)

        y1p = psum((128, 256))
        nc.tensor.matmul(y1p, r(W['m_W1T']), r(f1), start=True, stop=True)
        y1 = act.tile([128, 256], dt.float32, tag="y1")
        nc.scalar.activation(out=y1, in_=y1p, func=AF.Prelu,
                             bias=W['m_t1'], scale=W['m_s1'], alpha=W['mp1_a'])
        y2p = psum((128, 256))
        nc.tensor.matmul(y2p, r(W['m_W2T']), r(y1), start=True, stop=True)
        y2 = act.tile([128, 256], dt.float32, tag="y2")
        nc.scalar.activation(out=y2, in_=y2p, func=AF.Prelu,
                             bias=W['m_t2'], scale=W['m_s2'], alpha=W['mp2_a'])
        cp = psum_c((1, 256))
        nc.tensor.matmul(cp, r(W['m_W3T']), r(y2), start=True, stop=True)
        conf = small.tile([1, 256], dt.float32, tag="conf")
        nc.scalar.activation(out=conf, in_=cp, func=AF.Identity, bias=W['m_b3'])
        nc.sync.dma_start(out=out_dram[sc:sc + 1, :], in_=conf)


def _compile(mm_np=False):
    key = ('k', mm_np)
    if key in _COMPILED:
        return _COMPILED[key]
    import concourse.bass as bass  # noqa: F401
    import concourse.tile as tile
    import concourse.mybir as mybir
    from concourse import bacc

    dt = mybir.dt
    nc = bacc.Bacc("TRN2", target_bir_lowering=False, debug=False,
                   num_devices=NCORES)
    io = {}

    def din(name, shape):
        io[name] = nc.dram_tensor(name, list(shape), dt.float32,
                                  kind="ExternalInput").ap()

    din('centerT', (NB, 3, 256))
    din('detrT', (NB, 9, 128, 256))
    din('langT', (NG, 128, 256))
    din('maskT', (64, NSEQ))
    din('fc_W1T', (9, 128, 128))
    for nm in ['fc_s1', 'fc_t1', 'pr1_a', 'fc_b2', 'm_s1', 'm_t1', 'm_s2',
               'm_t2', 'm_W3T', 'mp1_a', 'mp2_a']:
        din(nm, (128, 1))
    for nm in ['fc_W2T', 'm_W1T', 'm_W2T']:
        din(nm, (128, 128))
    din('m_b3', (1, 1))
    for pre in ['sa', 'ca']:
        for i in range(DEPTH):
            p = f'{pre}{i}'
            for nm in ['WqT', 'WkT', 'WvT', 'WoT']:
                din(f'{p}_{nm}', (128, 128))
            for nm in ['bq', 'bo']:
                din(f'{p}_{nm}', (128, 1))
            for nm in ['lg', 'lb']:
                din(f'{p}_{nm}', (1, 128))
    io['out'] = nc.dram_tensor('out', [NSEQ, 256], dt.float32,
                               kind="ExternalOutput").ap()

    with tile.TileContext(nc) as tc:
        with ExitStack() as ctx:
            _build(nc, tc, ctx, io, mm_np=mm_np)
    nc.compile()
    _COMPILED[key] = (nc, list(io.keys()))
    return _COMPILED[key]


def kernel(**inputs) -> np.ndarray:
    from concourse.bass_utils import run_bass_kernel_spmd

    g, shards = _prep_host(inputs)
    nc, _names = _compile(mm_np=os.environ.get('KMM_F32') == '1')
    in_maps = []
    for c in range(NCORES):
        m = dict(shards[c])
        for k, v in g.items():
            m[k] = v
        in_maps.append(m)
    res = run_bass_kernel_spmd(nc, in_maps, core_ids=list(range(NCORES)),
                               trace=os.environ.get('KTRACE') == '1')
    out = np.zeros((B * M, K), np.float32)
    for c in range(NCORES):
        out[c * NSEQ:(c + 1) * NSEQ] = res.results[c]['out']
    kernel._last_results = res
    return out
